# revision 82
# baseline (speedup 1.0000x reference)
"""Trainium2 Bass kernel for GQA attention (b=2, s=2048, dim=1024, 16 q / 4 kv heads).

Sharding: 8 cores = 2 (batch) x 4 (head groups). Each core owns one batch
element and 4 q-heads + 1 kv-head (Wq/Wk/Wv column-sharded, Wo row-sharded).
Host pre-transposes everything contraction-major in bf16; host sums the 4
Wo partials per batch element in fp32.

v4 structure (v3 + startup/tail optimizations):
  - loops: head-pair hp (outer) -> tq chunk c of 512 -> tk tile j.
  - scores for both heads of a pair in ONE [128, 2, 512] fp32 PSUM tile
    (row-tiled concurrent matmuls at tile_position (0,0)/(64,0)); exp is a
    single ScalarE instruction per (hp, c, j); ScalarE runs ONLY exp.
  - causal diag mask applied ON THE PE: an extra accumulate-matmul adds a
    strictly-lower-triangular -60000 constant (ida stationary, ltri moving)
    into the scores PSUM before exp -> no cross-engine hop in the chain.
  - per-j emission order: exp(j) | fillers | scores(j+1) | PV(j), so the PE
    FIFO always has filler work while exp(j) runs and PV(j) never blocks
    the next scores. scores(c+1,0) is emitted before completion(c).
  - fillers are micro-tasks (2 matmuls each) scheduled so every input is
    produced a chunk ahead of first use and only DMA-landed data is touched.
  - normalization transpose path entirely in bf16 (pvs/pT/rec4/anrm).
  - startup: host layouts give 6-8KB contiguous DMA runs; the critical set
    is split across the Sync AND Scalar HWDGE issue queues in global
    priority order (both queues share 16 DMA engines, so issue order is
    priority order; the n1 prefetch comes last); dummy warmup matmuls keep
    the PE HAM-unthrottled while weights land; kv-chain emitted before q01
    so its serial evict->rope->krep chain overlaps q01's matmuls.
  - tail: the last chunk's Wo is split -- the attnT[0] halves run early as
    (1,2) fillers staged in fp16, the tail only does the attnT[1] matmul +
    add + DMA per tile.
"""

import sys
from contextlib import ExitStack

for _p in ("/opt/trn_rl_repo",):
    if _p not in sys.path:
        sys.path.insert(0, _p)

import numpy as np
import ml_dtypes

BF16 = ml_dtypes.bfloat16

P = 128
S = 2048          # sequence length
DIM = 1024        # model dim
HD = 64           # head dim
NT = S // P       # 16 token tiles
N_CT = DIM // P   # 8 contraction tiles for qkv proj
QKV = 384         # per-core projection rows: 256 q + 64 k + 64 v
CW = 512          # tq chunk width
NCH = S // CW     # 4 chunks

_NC_CACHE = {}


def _build_kernel_program():
    import concourse.bass as bass
    import concourse.tile as tile
    from concourse import bacc, mybir

    dt = mybir.dt
    f32, bf16 = dt.float32, dt.bfloat16
    AF = mybir.ActivationFunctionType

    nc = bacc.Bacc("TRN2", target_bir_lowering=False, debug=False)

    # host-prepped layouts chosen for maximal DMA contiguity: xt has 8KB
    # contiguous per (partition, chunk), wqkv 6KB per partition, outp tiles
    # are fully contiguous 128KB blocks
    xt = nc.dram_tensor("xt", [P, NCH, N_CT, CW], bf16, kind="ExternalInput").ap()
    wqkv = nc.dram_tensor("wqkv", [P, N_CT, QKV], bf16, kind="ExternalInput").ap()
    wo = nc.dram_tensor("wo", [256, DIM], bf16, kind="ExternalInput").ap()
    cost = nc.dram_tensor("cost", [P, S], bf16, kind="ExternalInput").ap()
    sint = nc.dram_tensor("sint", [P, S], bf16, kind="ExternalInput").ap()
    rott = nc.dram_tensor("rott", [P, P], bf16, kind="ExternalInput").ap()
    ident = nc.dram_tensor("ident", [P, P], bf16, kind="ExternalInput").ap()
    ltri = nc.dram_tensor("ltri", [P, P], bf16, kind="ExternalInput").ap()
    ida = nc.dram_tensor("ida", [P, P], bf16, kind="ExternalInput").ap()
    outp = nc.dram_tensor("outp", [NT, 2, P, CW], bf16, kind="ExternalOutput").ap()

    with tile.TileContext(nc) as tc:
        with ExitStack() as ctx:
            _emit(ctx, tc, nc, mybir, bass, dict(
                xt=xt, wqkv=wqkv, wo=wo, cost=cost, sint=sint, rott=rott,
                ident=ident, ltri=ltri, ida=ida, outp=outp,
            ), f32, bf16, AF)
    nc.compile()
    return nc


def _emit(ctx, tc, nc, mybir, bass, io, f32, bf16, AF):
    tp = tc.tile_pool

    const = ctx.enter_context(tp(name="const", bufs=1))
    persist = ctx.enter_context(tp(name="persist", bufs=1))
    tmp = ctx.enter_context(tp(name="tmp", bufs=4))
    ptp = ctx.enter_context(tp(name="pt", bufs=4))
    # PSUM pools: total exactly 16KB/partition (8 banks)
    scp = ctx.enter_context(tp(name="sc", bufs=2, space="PSUM"))   # 2x 4KB
    pvp = ctx.enter_context(tp(name="pv", bufs=2, space="PSUM"))   # 2x 2KB
    fil = ctx.enter_context(tp(name="fil", bufs=2, space="PSUM"))  # 2x 2KB

    # ---- DMA in first-need order, split across BOTH issue queues ----
    # DMA_DIRECT2D costs ~600ns of serial issue time per descriptor; the two
    # HWDGE queues (Sync + Scalar) share the same 16 DMA engines, so GLOBAL
    # issue order is priority order and the n1 prefetch comes after
    # everything startup-critical.
    wqkv_sb = persist.tile([P, N_CT, QKV], bf16, name="wqkv_sb", tag="wqkv_sb")
    # xt_sb indexed [partition, chunk, ct, token-within-chunk]
    xt_sb = persist.tile([P, NCH, N_CT, CW], bf16, name="xt_sb", tag="xt_sb")
    cost_sb = persist.tile([P, S], bf16, name="cost_sb", tag="cost_sb")
    sint_sb = persist.tile([P, S], bf16, name="sint_sb", tag="sint_sb")
    rott_sb = const.tile([P, P], bf16, tag="rott")
    ida_sb = const.tile([P, P], bf16, tag="ida")
    ltri_sb = const.tile([P, P], bf16, tag="ltri")
    ident_sb = const.tile([P, P], bf16, tag="ident")
    nc.scalar.dma_start(ida_sb[:], io["ida"])          # warmup dummies need it
    nc.sync.dma_start(xt_sb[:, 0, 0:2, :], io["xt"][:, 0, 0:2, :])
    nc.scalar.dma_start(wqkv_sb[:, 0:4, :], io["wqkv"][:, 0:4, :])
    nc.sync.dma_start(xt_sb[:, 0, 2:N_CT, :], io["xt"][:, 0, 2:N_CT, :])
    nc.scalar.dma_start(wqkv_sb[:, 4:N_CT, :], io["wqkv"][:, 4:N_CT, :])
    nc.scalar.dma_start(rott_sb[:], io["rott"])
    nc.sync.dma_start(cost_sb[:, 0:CW], io["cost"][:, 0:CW])
    nc.sync.dma_start(sint_sb[:, 0:CW], io["sint"][:, 0:CW])
    nc.scalar.dma_start(ltri_sb[:], io["ltri"])
    nc.scalar.dma_start(ident_sb[:], io["ident"])
    # token chunk n1 after the n0-critical set so c0-era fillers can start
    # projecting n1 on time; n2/n3/wo are DMA'd later from filler tasks
    nc.sync.dma_start(xt_sb[:, 1, :, :], io["xt"][:, 1, :, :])
    nc.sync.dma_start(cost_sb[:, CW:2 * CW], io["cost"][:, CW:2 * CW])
    nc.sync.dma_start(sint_sb[:, CW:2 * CW], io["sint"][:, CW:2 * CW])
    wo_sb = persist.tile([P, 2, DIM], bf16, name="wo_sb", tag="wo_sb")

    def dma_chunk_task(n):
        def f():
            sl = slice(n * CW, (n + 1) * CW)
            nc.sync.dma_start(xt_sb[:, n, :, :], io["xt"][:, n, :, :])
            nc.sync.dma_start(cost_sb[:, sl], io["cost"][:, sl])
            nc.sync.dma_start(sint_sb[:, sl], io["sint"][:, sl])
        return [f]

    def dma_wo_task():
        def f():
            nc.sync.dma_start(wo_sb[:], io["wo"].rearrange("(a p) e -> p a e", p=P))
        return [f]

    # ---- persistent SBUF activations ----
    q01T = persist.tile([P, S], bf16, name="q01T", tag="q01T")
    q23T = persist.tile([P, S], bf16, name="q23T", tag="q23T")
    kvT = persist.tile([P, S], bf16, name="kvT", tag="kvT")
    q01r = persist.tile([P, S], bf16, name="q01r", tag="q01r")
    q23r = persist.tile([P, S], bf16, name="q23r", tag="q23r")
    krep = persist.tile([P, S], bf16, name="krep", tag="krep")
    v_sb = persist.tile([P, NT, HD + 1], bf16, name="v_sb", tag="v_sb")
    nc.vector.memset(v_sb[:, :, 0:1], 1.0)
    attnT = [persist.tile([P, S], bf16, name="attnT01", tag="attnT01"),
             persist.tile([P, S], bf16, name="attnT23", tag="attnT23")]
    qrs = [q01r, q23r]

    # ---- micro-task fillers (each ~2 matmuls of PE work or less) ----
    qkv_dst = {"q01": (q01T, 0), "q23": (q23T, P), "kv": (kvT, 2 * P)}

    def proj_tasks(dst_name, n):
        """5 micro-tasks: 4x (2 ct-matmuls), 1x evict"""
        dst, mt = qkv_dst[dst_name]
        sl = slice(n * CW, (n + 1) * CW)
        st = {}

        def mk_mm(k):
            def f():
                if k == 0:
                    st["ps"] = fil.tile([P, CW], f32, name="ps", tag="fil")
                for cti in (2 * k, 2 * k + 1):
                    nc.tensor.matmul(
                        st["ps"], wqkv_sb[:, cti, mt:mt + P], xt_sb[:, n, cti, :],
                        start=(cti == 0), stop=(cti == N_CT - 1),
                    )
            return f

        def ev():
            nc.vector.tensor_copy(dst[:, sl], st["ps"])
        return [mk_mm(k) for k in range(4)] + [ev]

    def rope_tasks(src, dst, rows, n):
        """2 micro-tasks: (rot-matmul + cos-mul), (sin-mul + add)"""
        sl = slice(n * CW, (n + 1) * CW)
        st = {}

        def t_a():
            st["psr"] = fil.tile([P, CW], f32, name="psr", tag="fil")[:rows, :]
            nc.tensor.matmul(st["psr"], rott_sb[:rows, :rows], src[:rows, sl],
                             start=True, stop=True)
            st["t1"] = tmp.tile([P, CW], bf16, name="ropet1", tag="rope")[:rows]
            nc.gpsimd.tensor_mul(st["t1"], src[:rows, sl], cost_sb[:rows, sl])

        def t_b():
            t2 = tmp.tile([P, CW], bf16, name="ropet2", tag="rope")[:rows]
            nc.vector.tensor_mul(t2, st["psr"], sint_sb[:rows, sl])
            nc.vector.tensor_add(dst[:rows, sl], st["t1"], t2)
        return [t_a, t_b]

    def krep_task(n):
        # replicate roped k to partitions 64:128 via PE (col-group 64) + DVE
        # evict -- an SBUF->SBUF DMA here would queue behind megabytes of
        # input DMA and stall every score matmul
        def f():
            sl = slice(n * CW, (n + 1) * CW)
            pk = fil.tile([P, CW], f32, name="pk", tag="fil")
            nc.tensor.matmul(pk[64:128, :], ida_sb[0:64, 0:HD], krep[0:64, sl],
                             start=True, stop=True, tile_position=(0, 64))
            nc.vector.tensor_copy(krep[64:128, sl], pk[64:128, :])
        return [f]

    def v_task(j):
        def f():
            pst = fil.tile([P, CW], bf16, name="pst", tag="fil")[:, :HD]
            nc.tensor.transpose(pst, kvT[64:128, j * P:(j + 1) * P],
                                ident_sb[64:128, 0:HD])
            nc.vector.tensor_copy(v_sb[:, j, 1:HD + 1], pst)
        return [f]

    def wo_task(tt, e):
        def f():
            osb = tmp.tile([P, CW], bf16, name="osb", tag="osb", bufs=3)
            po = fil.tile([P, CW], f32, name="po", tag="fil")
            nc.tensor.matmul(po, attnT[0][:, tt * P:(tt + 1) * P],
                             wo_sb[:, 0, e * CW:(e + 1) * CW],
                             start=True, stop=False)
            nc.tensor.matmul(po, attnT[1][:, tt * P:(tt + 1) * P],
                             wo_sb[:, 1, e * CW:(e + 1) * CW],
                             start=False, stop=True)
            nc.vector.tensor_copy(osb[:], po)
            nc.sync.dma_start(io["outp"][tt, e, :, :], osb[:])
        return [f]

    # last-chunk wo split: the attnT[0] half runs early (hp0 long finished)
    # staged in fp16; the tail only does the attnT[1] matmul + add + DMA
    f16 = mybir.dt.float16
    wop_sb = persist.tile([P, 8, CW], f16, name="wop_sb", tag="wop_sb")

    def wo_early(tt, e):
        def f():
            po = fil.tile([P, CW], f32, name="po", tag="fil")
            nc.tensor.matmul(po, attnT[0][:, tt * P:(tt + 1) * P],
                             wo_sb[:, 0, e * CW:(e + 1) * CW],
                             start=True, stop=True)
            nc.vector.tensor_copy(wop_sb[:, (tt % 4) * 2 + e, :], po)
        return [f]

    def wo_late(tt, e):
        def f():
            osb = tmp.tile([P, CW], bf16, name="osb", tag="osb", bufs=3)
            po = fil.tile([P, CW], f32, name="po", tag="fil")
            nc.tensor.matmul(po, attnT[1][:, tt * P:(tt + 1) * P],
                             wo_sb[:, 1, e * CW:(e + 1) * CW],
                             start=True, stop=True)
            nc.vector.tensor_add(osb[:], po, wop_sb[:, (tt % 4) * 2 + e, :])
            nc.sync.dma_start(io["outp"][tt, e, :, :], osb[:])
        return [f]

    def kv_chain(n):
        # kv proj + k-rope + krep for token chunk n (scores j>=4n need krep)
        return (proj_tasks("kv", n) + rope_tasks(kvT, krep, HD, n)
                + krep_task(n))

    def v_chain(n):
        t = []
        for jj in range(4 * n, 4 * n + 4):
            t += v_task(jj)
        return t

    def wo_chunk(c):
        t = []
        for tt in range(4 * c, 4 * c + 4):
            for e in range(2):
                t += wo_task(tt, e)
        return t

    # schedule: (hp, c) -> (filler list, per-j budget). Every producer runs
    # at least one chunk before its consumer; kv/krep/v for chunk c+1 are
    # produced early inside chunk c+1 itself (consumed before j reaches 4c+4).
    # schedule: every producer runs at least one chunk before its consumer;
    # kv/krep/v for chunk c+1 are produced early inside chunk c+1 itself
    # (consumed before j reaches 4c+4)
    sched = {
        # (0,0): DMA-independent work first (q23 n0 uses resident xt n0) so
        # the in-order PE FIFO never stalls on the xt-n1 DMA during the
        # first chunk; q01-n1 tasks go last (run near chunk end, DMA landed)
        # third element: where the previous chunk's deferred completion
        # tasks are inserted (after the kv chain for hp0 chunks)
        (0, 0): (proj_tasks("q01", 1)
                 + rope_tasks(q01T, q01r, P, 1), 3, 0),
        # q01 chains drain BEFORE the v tasks: the next chunk's first scores
        # depends on the q01 rope, and the surplus fillers that discharge at
        # the chunk boundary must not delay it in the PE FIFO
        (0, 1): (dma_chunk_task(3) + kv_chain(1)
                 + proj_tasks("q01", 2) + rope_tasks(q01T, q01r, P, 2)
                 + v_chain(1)
                 + proj_tasks("q23", 1) + rope_tasks(q23T, q23r, P, 1), 4, 9),
        (0, 2): (dma_wo_task() + kv_chain(2) + v_chain(2)
                 + proj_tasks("q01", 3) + rope_tasks(q01T, q01r, P, 3), 2, 9),
        (0, 3): (kv_chain(3) + v_chain(3), 1, 8),
        # q23 chains for the last two hp1 chunks trickle through hp1 at
        # budget 1: keeps hp1's per-j PE near the exp cadence (denser for
        # HAM) without delaying critical ops
        (1, 0): (proj_tasks("q23", 2), 1, 0),
        (1, 1): (rope_tasks(q23T, q23r, P, 2)
                 + proj_tasks("q23", 3), 1, 0),
        (1, 2): (rope_tasks(q23T, q23r, P, 3) + wo_chunk(0)
                 + [t for tt in range(12, 16) for e in range(2)
                    for t in wo_early(tt, e)], 2, 0),
        (1, 3): (wo_chunk(1) + wo_chunk(2), 1, 0),
    }

    # ---- preamble: tokens 0:512 projected + roped (critical path) ----
    # PE warmup during the DMA wait: dummy matmuls on already-landed tiles
    # keep the PE busy so HAM un-throttles before the real preamble
    for _w in range(6):
        wps = scp.tile([P, 2, CW], f32, name="warm", tag="sc")
        nc.tensor.matmul(wps[:, 0, :], ida_sb, xt_sb[:, 0, 0, :],
                         start=True, stop=True)
    # kv proj then q01 proj back-to-back on the PE; kv's serial cross-engine
    # chain (evict -> rope -> krep) overlaps q01's matmuls.  q23-n0's proj
    # fills the PE while the rope/krep DVE chains run so HAM never
    # re-throttles before the first chunk's j-loop.
    kvc = kv_chain(0)
    q01c = proj_tasks("q01", 0) + rope_tasks(q01T, q01r, P, 0)
    q23c = proj_tasks("q23", 0) + rope_tasks(q23T, q23r, P, 0)
    for t in (kvc[:5] + q01c[:5] + kvc[5:6] + q23c[:5] + kvc[6:]
              + q01c[5:] + q23c[5:] + v_chain(0)):
        t()
    # n2 prefetch issued here (pure DMA, no PE work) so (0,0)'s filler list
    # is short enough that the q01-n1 rope drains by j=2 and scores(0,1,0)
    # never waits at the transition
    for t in dma_chunk_task(2):
        t()

    def scores(hp, c, j):
        """S^T for both heads of pair hp, tk tile j, tq chunk c.
        Diagonal j also accumulates a -60000 strictly-lower-tri block so the
        later exp zeroes masked positions (PE-side masking)."""
        lo = max(0, j * P - c * CW)
        diag = j >= 4 * c
        sc = scp.tile([P, 2, CW], f32, name="sc", tag="sc")
        for h in range(2):
            nc.tensor.matmul(
                sc[:, h, lo:CW], krep[64 * h:64 * h + 64, j * P:(j + 1) * P],
                qrs[hp][64 * h:64 * h + 64, c * CW + lo:(c + 1) * CW],
                start=True, stop=not diag, tile_position=(64 * h, 0),
            )
        if diag:
            for h in range(2):
                nc.tensor.matmul(
                    sc[:, h, lo:lo + P], ida_sb, ltri_sb,
                    start=False, stop=True, skip_group_check=True,
                )
        return sc, lo

    def comp_h(hp, c, h, pvs_sb):
        """Deferred half of the chunk-completion: transposes + recip + mul +
        transpose-back + attnT copy for one head.  Self-contained fil-pool
        lifetimes so it can interleave with any other filler task."""
        def f():
            gcol = c * CW
            rec4 = tmp.tile([P, 4], bf16, name="rec4", tag="rec4", bufs=4)
            anrm = tmp.tile([P, 4, HD], bf16, name="anrm", tag="anrm", bufs=4)
            pT = fil.tile([P, 264], bf16, name="pT", tag="fil")
            for b in range(4):
                nc.tensor.transpose(
                    pT[:, 66 * b:66 * b + 65],
                    pvs_sb[0:HD + 1, h, 128 * b:128 * (b + 1)],
                    ida_sb[0:HD + 1, 0:HD + 1])
            pT3 = pT.rearrange("p (b c) -> p b c", c=66)
            with nc.allow_low_precision(reason="bf16 softmax denom recip"):
                nc.vector.reciprocal(rec4[:, :], pT3[:, :, 0])
            nc.vector.tensor_mul(
                anrm[:, :, :], pT3[:, :, 1:HD + 1],
                rec4[:, :, None].broadcast_to([P, 4, HD]))
            pout = fil.tile([P, CW], bf16, name="pout", tag="fil")
            tpos = (0, 0) if h == 0 else (0, HD)
            rows = slice(0, HD) if h == 0 else slice(HD, P)
            for b in range(4):
                nc.tensor.transpose(
                    pout[rows, 128 * b:128 * (b + 1)],
                    anrm[:, b, :], ida_sb[:, :], tile_position=tpos)
            nc.vector.tensor_copy(attnT[hp][rows, gcol:gcol + CW],
                                  pout[rows, 0:CW])
        return f

    first = True
    sc_cur = lo_cur = None
    deferred = []
    for hp in range(2):
        for c in range(NCH):
            jmax = 4 * c + 3
            base_fillers, budget, ins = sched[(hp, c)]
            # deferred completion drains early (hp1 wo tasks read its attnT
            # output) but AFTER this chunk's kv/krep chain so the scores it
            # feeds are never displaced
            fillers = base_fillers[:ins] + deferred + base_fillers[ins:]
            deferred = []
            fi = 0
            if first:
                sc_cur, lo_cur = scores(0, 0, 0)
                first = False
            pvE = pvp.tile([P, CW], f32, name="pvE", tag="pv")
            pvO = pvp.tile([P, CW], f32, name="pvO", tag="pv")
            for j in range(jmax + 1):
                sc, lo = sc_cur, lo_cur
                pt = ptp.tile([P, 2, CW], bf16, name="pt", tag="pt")
                nc.scalar.activation(pt[:, :, lo:CW], sc[:, :, lo:CW],
                                     AF.Exp, scale=0.125)
                for _ in range(budget):
                    if fi < len(fillers):
                        fillers[fi]()
                        fi += 1
                if j < jmax:
                    sc_cur, lo_cur = scores(hp, c, j + 1)
                st, sp = (j == 0), (j == jmax)
                nc.tensor.matmul(pvE[0:HD + 1, lo:CW], v_sb[:, j, :],
                                 pt[:, 0, lo:CW], start=st, stop=sp)
                nc.tensor.matmul(pvO[0:HD + 1, lo:CW], v_sb[:, j, :],
                                 pt[:, 1, lo:CW], start=st, stop=sp)
            while fi < len(fillers):
                fillers[fi]()
                fi += 1
            # next chunk's first scores ahead of the completion chain
            if (hp, c) != (1, NCH - 1):
                nhp, ncc = (hp, c + 1) if c < NCH - 1 else (hp + 1, 0)
                sc_cur, lo_cur = scores(nhp, ncc, 0)
            # ---- chunk complete: evict pv to SBUF inline (the only pv
            # readers), then DEFER the transpose-normalize work into the
            # next chunk's filler stream so it never serializes the
            # inter-chunk PE FIFO
            pvs_sb = tmp.tile([P, 2, CW], bf16, name="pvs", tag="pvs", bufs=2)
            nc.vector.tensor_copy(pvs_sb[0:HD + 1, 0, :], pvE[0:HD + 1, :])
            nc.vector.tensor_copy(pvs_sb[0:HD + 1, 1, :], pvO[0:HD + 1, :])
            if (hp, c) != (1, NCH - 1):
                deferred = [comp_h(hp, c, 0, pvs_sb), comp_h(hp, c, 1, pvs_sb)]
            else:
                comp_h(hp, c, 0, pvs_sb)()
                comp_h(hp, c, 1, pvs_sb)()

    # ---- tail: only the attnT[1]-half of the last chunk's Wo remains ----
    for tt in range(12, 16):
        for e in range(2):
            for t in wo_late(tt, e):
                t()


def _host_inputs(X, cos, sin, Wq, Wk, Wv, Wo):
    """Build the 8 per-core input maps (host-side sharding + layout prep)."""
    cosT = np.ascontiguousarray(cos.T)  # [64, 2048]
    sinT = np.ascontiguousarray(sin.T)
    cost = np.concatenate([cosT, cosT], 0).astype(BF16)  # [128, 2048]
    sint = np.concatenate([sinT, sinT], 0).astype(BF16)
    rott = np.zeros((P, P), np.float32)
    idx = np.arange(0, P, 2)
    rott[idx, idx + 1] = 1.0    # RT[2i, 2i+1] = +1
    rott[idx + 1, idx] = -1.0   # RT[2i+1, 2i] = -1
    rott = rott.astype(BF16)
    ident = np.zeros((P, P), np.float32)
    ident[0:64, 0:64] = np.eye(64)
    ident[64:128, 0:64] = np.eye(64)   # same I64 available at base partition 64
    ident = ident.astype(BF16)
    # strictly-lower-triangular -60000: added into scores before exp so the
    # upper-left (tk > tq) of each diagonal block becomes exp(-inf) = 0
    ltri = np.tril(np.full((P, P), -60000.0, np.float32), k=-1).astype(BF16)
    ida = np.eye(P, dtype=np.float32).astype(BF16)

    # xt host layout [p, chunk, ct, t]: 8KB contiguous per (p, chunk) DMA run
    xts = [
        np.ascontiguousarray(
            X[b].T.astype(BF16).reshape(8, P, 4, CW).transpose(1, 2, 0, 3))
        for b in range(X.shape[0])
    ]

    in_maps = []
    for c in range(8):
        b, g = c // 4, c % 4
        wqkv = np.concatenate(
            [Wq[256 * g:256 * (g + 1)], Wk[64 * g:64 * (g + 1)], Wv[64 * g:64 * (g + 1)]], 0
        ).T.astype(BF16)                                   # [1024, 384]
        # wqkv host layout [p, ct, d]: 6KB contiguous per partition
        wqkv = np.ascontiguousarray(wqkv.reshape(8, P, QKV).transpose(1, 0, 2))
        wog = np.ascontiguousarray(Wo[:, 256 * g:256 * (g + 1)].T).astype(BF16)  # [256, 1024]
        in_maps.append({
            "xt": xts[b], "wqkv": wqkv, "wo": wog,
            "cost": cost, "sint": sint, "rott": rott, "ident": ident,
            "ltri": ltri, "ida": ida,
        })
    return in_maps


def get_nc():
    if "nc" not in _NC_CACHE:
        _NC_CACHE["nc"] = _build_kernel_program()
    return _NC_CACHE["nc"]


def _install_ntff_hook():
    """The agent image's antenv lacks axon_hooks; recreate it so trace=True
    can reach the terminal's NRT profiler (timing only, not needed for
    correctness)."""
    import types
    if "antenv.axon_hooks" in sys.modules:
        return
    try:
        import antenv
        m = types.ModuleType("antenv.axon_hooks")
        holder = {"v": None}
        m.set_axon_ntff_profile_hook = lambda h: holder.__setitem__("v", h)
        m.get_axon_ntff_profile_hook = lambda: holder["v"]
        sys.modules["antenv.axon_hooks"] = m
        antenv.axon_hooks = m
        from trn_agent_boot.trn_boot import _ntff_profile_via_ctypes
        m.set_axon_ntff_profile_hook(
            _ntff_profile_via_ctypes("/opt/axon/libaxon_pjrt.so"))
    except Exception:
        pass


def kernel(X, freqs_cos, freqs_sin, Wq, Wk, Wv, Wo, _trace=False):
    from concourse.bass_utils import run_bass_kernel_spmd

    if _trace:
        _install_ntff_hook()

    X = np.asarray(X, np.float32)
    in_maps = _host_inputs(
        X, np.asarray(freqs_cos, np.float32), np.asarray(freqs_sin, np.float32),
        np.asarray(Wq, np.float32), np.asarray(Wk, np.float32),
        np.asarray(Wv, np.float32), np.asarray(Wo, np.float32),
    )
    nc = get_nc()
    res = run_bass_kernel_spmd(nc, in_maps, core_ids=list(range(8)), trace=_trace)
    out = np.zeros((2, S, DIM), np.float32)
    for c in range(8):
        o = res.results[c]["outp"].astype(np.float32)   # [16, 2, 128, 512]
        out[c // 4] += o.transpose(0, 2, 1, 3).reshape(S, DIM)
    if _trace:
        kernel.last_result = res
    return out



# revision 83
# speedup vs baseline: 1.0166x; 1.0166x over previous
"""Trainium2 Bass kernel for GQA attention (b=2, s=2048, dim=1024, 16 q / 4 kv heads).

Sharding: 8 cores = 2 (batch) x 4 (head groups). Each core owns one batch
element and 4 q-heads + 1 kv-head (Wq/Wk/Wv column-sharded, Wo row-sharded).
Host pre-transposes everything contraction-major in bf16; host sums the 4
Wo partials per batch element in fp32.

v4 structure (v3 + startup/tail optimizations):
  - loops: head-pair hp (outer) -> tq chunk c of 512 -> tk tile j.
  - scores for both heads of a pair in ONE [128, 2, 512] fp32 PSUM tile
    (row-tiled concurrent matmuls at tile_position (0,0)/(64,0)); exp is a
    single ScalarE instruction per (hp, c, j); ScalarE runs ONLY exp.
  - causal diag mask applied ON THE PE: an extra accumulate-matmul adds a
    strictly-lower-triangular -60000 constant (ida stationary, ltri moving)
    into the scores PSUM before exp -> no cross-engine hop in the chain.
  - per-j emission order: exp(j) | fillers | scores(j+1) | PV(j), so the PE
    FIFO always has filler work while exp(j) runs and PV(j) never blocks
    the next scores. scores(c+1,0) is emitted before completion(c).
  - fillers are micro-tasks (2 matmuls each) scheduled so every input is
    produced a chunk ahead of first use and only DMA-landed data is touched.
  - normalization transpose path entirely in bf16 (pvs/pT/rec4/anrm).
  - startup: host layouts give 6-8KB contiguous DMA runs; the critical set
    is split across the Sync AND Scalar HWDGE issue queues in global
    priority order (both queues share 16 DMA engines, so issue order is
    priority order; the n1 prefetch comes last); dummy warmup matmuls keep
    the PE HAM-unthrottled while weights land; kv-chain emitted before q01
    so its serial evict->rope->krep chain overlaps q01's matmuls.
  - tail: the last chunk's Wo is split -- the attnT[0] halves run early as
    (1,2) fillers staged in fp16, the tail only does the attnT[1] matmul +
    add + DMA per tile.
"""

import sys
from contextlib import ExitStack

for _p in ("/opt/trn_rl_repo",):
    if _p not in sys.path:
        sys.path.insert(0, _p)

import numpy as np
import ml_dtypes

BF16 = ml_dtypes.bfloat16

P = 128
S = 2048          # sequence length
DIM = 1024        # model dim
HD = 64           # head dim
NT = S // P       # 16 token tiles
N_CT = DIM // P   # 8 contraction tiles for qkv proj
QKV = 384         # per-core projection rows: 256 q + 64 k + 64 v
CW = 512          # tq chunk width
NCH = S // CW     # 4 chunks

_NC_CACHE = {}


def _build_kernel_program():
    import concourse.bass as bass
    import concourse.tile as tile
    from concourse import bacc, mybir

    dt = mybir.dt
    f32, bf16 = dt.float32, dt.bfloat16
    AF = mybir.ActivationFunctionType

    nc = bacc.Bacc("TRN2", target_bir_lowering=False, debug=False)

    # host-prepped layouts chosen for maximal DMA contiguity: xt has 8KB
    # contiguous per (partition, chunk), wqkv 6KB per partition, outp tiles
    # are fully contiguous 128KB blocks
    xt = nc.dram_tensor("xt", [P, NCH, N_CT, CW], bf16, kind="ExternalInput").ap()
    wqkv = nc.dram_tensor("wqkv", [P, N_CT, QKV], bf16, kind="ExternalInput").ap()
    wo = nc.dram_tensor("wo", [256, DIM], bf16, kind="ExternalInput").ap()
    cost = nc.dram_tensor("cost", [P, S], bf16, kind="ExternalInput").ap()
    sint = nc.dram_tensor("sint", [P, S], bf16, kind="ExternalInput").ap()
    rott = nc.dram_tensor("rott", [P, P], bf16, kind="ExternalInput").ap()
    ident = nc.dram_tensor("ident", [P, P], bf16, kind="ExternalInput").ap()
    ltri = nc.dram_tensor("ltri", [P, P], bf16, kind="ExternalInput").ap()
    ida = nc.dram_tensor("ida", [P, P], bf16, kind="ExternalInput").ap()
    outp = nc.dram_tensor("outp", [NT, 2, P, CW], bf16, kind="ExternalOutput").ap()

    with tile.TileContext(nc) as tc:
        with ExitStack() as ctx:
            _emit(ctx, tc, nc, mybir, bass, dict(
                xt=xt, wqkv=wqkv, wo=wo, cost=cost, sint=sint, rott=rott,
                ident=ident, ltri=ltri, ida=ida, outp=outp,
            ), f32, bf16, AF)
    nc.compile()
    return nc


def _emit(ctx, tc, nc, mybir, bass, io, f32, bf16, AF):
    tp = tc.tile_pool

    const = ctx.enter_context(tp(name="const", bufs=1))
    persist = ctx.enter_context(tp(name="persist", bufs=1))
    tmp = ctx.enter_context(tp(name="tmp", bufs=4))
    ptp = ctx.enter_context(tp(name="pt", bufs=4))
    # PSUM pools: total exactly 16KB/partition (8 banks)
    scp = ctx.enter_context(tp(name="sc", bufs=2, space="PSUM"))   # 2x 4KB
    pvp = ctx.enter_context(tp(name="pv", bufs=2, space="PSUM"))   # 2x 2KB
    fil = ctx.enter_context(tp(name="fil", bufs=2, space="PSUM"))  # 2x 2KB

    # ---- DMA in first-need order, split across BOTH issue queues ----
    # DMA_DIRECT2D costs ~600ns of serial issue time per descriptor; the two
    # HWDGE queues (Sync + Scalar) share the same 16 DMA engines, so GLOBAL
    # issue order is priority order and the n1 prefetch comes after
    # everything startup-critical.
    wqkv_sb = persist.tile([P, N_CT, QKV], bf16, name="wqkv_sb", tag="wqkv_sb")
    # xt_sb indexed [partition, chunk, ct, token-within-chunk]
    xt_sb = persist.tile([P, NCH, N_CT, CW], bf16, name="xt_sb", tag="xt_sb")
    cost_sb = persist.tile([P, S], bf16, name="cost_sb", tag="cost_sb")
    sint_sb = persist.tile([P, S], bf16, name="sint_sb", tag="sint_sb")
    rott_sb = const.tile([P, P], bf16, tag="rott")
    ida_sb = const.tile([P, P], bf16, tag="ida")
    ltri_sb = const.tile([P, P], bf16, tag="ltri")
    ident_sb = const.tile([P, P], bf16, tag="ident")
    nc.scalar.dma_start(ida_sb[:], io["ida"])          # warmup dummies need it
    nc.sync.dma_start(xt_sb[:, 0, 0:2, :], io["xt"][:, 0, 0:2, :])
    nc.scalar.dma_start(wqkv_sb[:, 0:4, :], io["wqkv"][:, 0:4, :])
    nc.sync.dma_start(xt_sb[:, 0, 2:N_CT, :], io["xt"][:, 0, 2:N_CT, :])
    nc.scalar.dma_start(wqkv_sb[:, 4:N_CT, :], io["wqkv"][:, 4:N_CT, :])
    nc.scalar.dma_start(rott_sb[:], io["rott"])
    nc.sync.dma_start(cost_sb[:, 0:CW], io["cost"][:, 0:CW])
    nc.sync.dma_start(sint_sb[:, 0:CW], io["sint"][:, 0:CW])
    nc.scalar.dma_start(ltri_sb[:], io["ltri"])
    nc.scalar.dma_start(ident_sb[:], io["ident"])
    # token chunk n1 after the n0-critical set so c0-era fillers can start
    # projecting n1 on time; n2/n3/wo are DMA'd later from filler tasks
    nc.sync.dma_start(xt_sb[:, 1, :, :], io["xt"][:, 1, :, :])
    nc.sync.dma_start(cost_sb[:, CW:2 * CW], io["cost"][:, CW:2 * CW])
    nc.sync.dma_start(sint_sb[:, CW:2 * CW], io["sint"][:, CW:2 * CW])
    wo_sb = persist.tile([P, 2, DIM], bf16, name="wo_sb", tag="wo_sb")

    def dma_chunk_task(n):
        def f():
            sl = slice(n * CW, (n + 1) * CW)
            nc.sync.dma_start(xt_sb[:, n, :, :], io["xt"][:, n, :, :])
            nc.sync.dma_start(cost_sb[:, sl], io["cost"][:, sl])
            nc.sync.dma_start(sint_sb[:, sl], io["sint"][:, sl])
        return [f]

    def dma_wo_task():
        def f():
            nc.sync.dma_start(wo_sb[:], io["wo"].rearrange("(a p) e -> p a e", p=P))
        return [f]

    # ---- persistent SBUF activations ----
    q01T = persist.tile([P, S], bf16, name="q01T", tag="q01T")
    q23T = persist.tile([P, S], bf16, name="q23T", tag="q23T")
    kvT = persist.tile([P, S], bf16, name="kvT", tag="kvT")
    q01r = persist.tile([P, S], bf16, name="q01r", tag="q01r")
    q23r = persist.tile([P, S], bf16, name="q23r", tag="q23r")
    krep = persist.tile([P, S], bf16, name="krep", tag="krep")
    v_sb = persist.tile([P, NT, HD + 1], bf16, name="v_sb", tag="v_sb")
    nc.vector.memset(v_sb[:, :, 0:1], 1.0)
    attnT = [persist.tile([P, S], bf16, name="attnT01", tag="attnT01"),
             persist.tile([P, S], bf16, name="attnT23", tag="attnT23")]
    qrs = [q01r, q23r]

    # ---- micro-task fillers (each ~2 matmuls of PE work or less) ----
    qkv_dst = {"q01": (q01T, 0), "q23": (q23T, P), "kv": (kvT, 2 * P)}

    def proj_tasks(dst_name, n):
        """5 micro-tasks: 4x (2 ct-matmuls), 1x evict"""
        dst, mt = qkv_dst[dst_name]
        sl = slice(n * CW, (n + 1) * CW)
        st = {}

        def mk_mm(k):
            def f():
                if k == 0:
                    st["ps"] = fil.tile([P, CW], f32, name="ps", tag="fil")
                for cti in (2 * k, 2 * k + 1):
                    nc.tensor.matmul(
                        st["ps"], wqkv_sb[:, cti, mt:mt + P], xt_sb[:, n, cti, :],
                        start=(cti == 0), stop=(cti == N_CT - 1),
                    )
            return f

        def ev():
            nc.vector.tensor_copy(dst[:, sl], st["ps"])
        return [mk_mm(k) for k in range(4)] + [ev]

    def rope_tasks(src, dst, rows, n):
        """2 micro-tasks: (rot-matmul + cos-mul), (sin-mul + add)"""
        sl = slice(n * CW, (n + 1) * CW)
        st = {}

        def t_a():
            st["psr"] = fil.tile([P, CW], f32, name="psr", tag="fil")[:rows, :]
            nc.tensor.matmul(st["psr"], rott_sb[:rows, :rows], src[:rows, sl],
                             start=True, stop=True)
            st["t1"] = tmp.tile([P, CW], bf16, name="ropet1", tag="rope")[:rows]
            nc.gpsimd.tensor_mul(st["t1"], src[:rows, sl], cost_sb[:rows, sl])

        def t_b():
            t2 = tmp.tile([P, CW], bf16, name="ropet2", tag="rope")[:rows]
            nc.vector.tensor_mul(t2, st["psr"], sint_sb[:rows, sl])
            nc.vector.tensor_add(dst[:rows, sl], st["t1"], t2)
        return [t_a, t_b]

    def krep_task(n):
        # replicate roped k to partitions 64:128 via PE (col-group 64) + DVE
        # evict -- an SBUF->SBUF DMA here would queue behind megabytes of
        # input DMA and stall every score matmul
        def f():
            sl = slice(n * CW, (n + 1) * CW)
            pk = fil.tile([P, CW], f32, name="pk", tag="fil")
            nc.tensor.matmul(pk[64:128, :], ida_sb[0:64, 0:HD], krep[0:64, sl],
                             start=True, stop=True, tile_position=(0, 64))
            nc.vector.tensor_copy(krep[64:128, sl], pk[64:128, :])
        return [f]

    def v_task(j):
        def f():
            pst = fil.tile([P, CW], bf16, name="pst", tag="fil")[:, :HD]
            nc.tensor.transpose(pst, kvT[64:128, j * P:(j + 1) * P],
                                ident_sb[64:128, 0:HD])
            nc.vector.tensor_copy(v_sb[:, j, 1:HD + 1], pst)
        return [f]

    def wo_task(tt, e):
        def f():
            osb = tmp.tile([P, CW], bf16, name="osb", tag="osb", bufs=3)
            po = fil.tile([P, CW], f32, name="po", tag="fil")
            nc.tensor.matmul(po, attnT[0][:, tt * P:(tt + 1) * P],
                             wo_sb[:, 0, e * CW:(e + 1) * CW],
                             start=True, stop=False)
            nc.tensor.matmul(po, attnT[1][:, tt * P:(tt + 1) * P],
                             wo_sb[:, 1, e * CW:(e + 1) * CW],
                             start=False, stop=True)
            nc.vector.tensor_copy(osb[:], po)
            nc.sync.dma_start(io["outp"][tt, e, :, :], osb[:])
        return [f]

    # last-chunk wo split: the attnT[0] half runs early (hp0 long finished)
    # staged in fp16; the tail only does the attnT[1] matmul + add + DMA
    f16 = mybir.dt.float16
    wop_sb = persist.tile([P, 8, CW], f16, name="wop_sb", tag="wop_sb")

    def wo_early(tt, e):
        def f():
            po = fil.tile([P, CW], f32, name="po", tag="fil")
            nc.tensor.matmul(po, attnT[0][:, tt * P:(tt + 1) * P],
                             wo_sb[:, 0, e * CW:(e + 1) * CW],
                             start=True, stop=True)
            nc.vector.tensor_copy(wop_sb[:, (tt % 4) * 2 + e, :], po)
        return [f]

    def wo_late(tt, e):
        def f():
            osb = tmp.tile([P, CW], bf16, name="osb", tag="osb", bufs=3)
            po = fil.tile([P, CW], f32, name="po", tag="fil")
            nc.tensor.matmul(po, attnT[1][:, tt * P:(tt + 1) * P],
                             wo_sb[:, 1, e * CW:(e + 1) * CW],
                             start=True, stop=True)
            nc.vector.tensor_add(osb[:], po, wop_sb[:, (tt % 4) * 2 + e, :])
            nc.sync.dma_start(io["outp"][tt, e, :, :], osb[:])
        return [f]

    def kv_chain(n):
        # kv proj + k-rope + krep for token chunk n (scores j>=4n need krep)
        return (proj_tasks("kv", n) + rope_tasks(kvT, krep, HD, n)
                + krep_task(n))

    def v_chain(n):
        t = []
        for jj in range(4 * n, 4 * n + 4):
            t += v_task(jj)
        return t

    def wo_chunk(c):
        t = []
        for tt in range(4 * c, 4 * c + 4):
            for e in range(2):
                t += wo_task(tt, e)
        return t

    # schedule: (hp, c) -> (filler list, per-j budget). Every producer runs
    # at least one chunk before its consumer; kv/krep/v for chunk c+1 are
    # produced early inside chunk c+1 itself (consumed before j reaches 4c+4).
    # schedule: every producer runs at least one chunk before its consumer;
    # kv/krep/v for chunk c+1 are produced early inside chunk c+1 itself
    # (consumed before j reaches 4c+4)
    sched = {
        # (0,0): DMA-independent work first (q23 n0 uses resident xt n0) so
        # the in-order PE FIFO never stalls on the xt-n1 DMA during the
        # first chunk; q01-n1 tasks go last (run near chunk end, DMA landed)
        # third element: where the previous chunk's deferred completion
        # tasks are inserted (after the kv chain for hp0 chunks)
        (0, 0): (proj_tasks("q01", 1)
                 + rope_tasks(q01T, q01r, P, 1), 3, 0),
        (0, 1): (dma_chunk_task(3) + kv_chain(1) + v_chain(1)
                 + proj_tasks("q01", 2) + rope_tasks(q01T, q01r, P, 2)
                 + proj_tasks("q23", 1) + rope_tasks(q23T, q23r, P, 1), 4, 9),
        (0, 2): (dma_wo_task() + kv_chain(2) + v_chain(2)
                 + proj_tasks("q01", 3) + rope_tasks(q01T, q01r, P, 3), 2, 9),
        (0, 3): (kv_chain(3) + v_chain(3), 1, 8),
        # q23 chains for the last two hp1 chunks trickle through hp1 at
        # budget 1: keeps hp1's per-j PE near the exp cadence (denser for
        # HAM) without delaying critical ops
        (1, 0): (proj_tasks("q23", 2), 1, 0),
        (1, 1): (rope_tasks(q23T, q23r, P, 2)
                 + proj_tasks("q23", 3), 1, 0),
        (1, 2): (rope_tasks(q23T, q23r, P, 3) + wo_chunk(0)
                 + [t for tt in range(12, 16) for e in range(2)
                    for t in wo_early(tt, e)], 2, 0),
        (1, 3): (wo_chunk(1) + wo_chunk(2), 1, 0),
    }

    # ---- preamble: tokens 0:512 projected + roped (critical path) ----
    # PE warmup during the DMA wait: dummy matmuls on already-landed tiles
    # keep the PE busy so HAM un-throttles before the real preamble
    for _w in range(6):
        wps = scp.tile([P, 2, CW], f32, name="warm", tag="sc")
        nc.tensor.matmul(wps[:, 0, :], ida_sb, xt_sb[:, 0, 0, :],
                         start=True, stop=True)
    # kv proj then q01 proj back-to-back on the PE; kv's serial cross-engine
    # chain (evict -> rope -> krep) overlaps q01's matmuls.  q23-n0's proj
    # fills the PE while the rope/krep DVE chains run so HAM never
    # re-throttles before the first chunk's j-loop.
    kvc = kv_chain(0)
    q01c = proj_tasks("q01", 0) + rope_tasks(q01T, q01r, P, 0)
    q23c = proj_tasks("q23", 0) + rope_tasks(q23T, q23r, P, 0)
    for t in (kvc[:5] + q01c[:5] + kvc[5:6] + q23c[:5] + kvc[6:]
              + q01c[5:] + q23c[5:] + v_chain(0)):
        t()
    # n2 prefetch issued here (pure DMA, no PE work) so (0,0)'s filler list
    # is short enough that the q01-n1 rope drains by j=2 and scores(0,1,0)
    # never waits at the transition
    for t in dma_chunk_task(2):
        t()

    def scores(hp, c, j):
        """S^T for both heads of pair hp, tk tile j, tq chunk c.
        Diagonal j also accumulates a -60000 strictly-lower-tri block so the
        later exp zeroes masked positions (PE-side masking)."""
        lo = max(0, j * P - c * CW)
        diag = j >= 4 * c
        sc = scp.tile([P, 2, CW], f32, name="sc", tag="sc")
        for h in range(2):
            nc.tensor.matmul(
                sc[:, h, lo:CW], krep[64 * h:64 * h + 64, j * P:(j + 1) * P],
                qrs[hp][64 * h:64 * h + 64, c * CW + lo:(c + 1) * CW],
                start=True, stop=not diag, tile_position=(64 * h, 0),
            )
        if diag:
            for h in range(2):
                nc.tensor.matmul(
                    sc[:, h, lo:lo + P], ida_sb, ltri_sb,
                    start=False, stop=True, skip_group_check=True,
                )
        return sc, lo

    def comp_h(hp, c, h, pvs_sb):
        """Deferred half of the chunk-completion: transposes + recip + mul +
        transpose-back + attnT copy for one head.  Self-contained fil-pool
        lifetimes so it can interleave with any other filler task."""
        def f():
            gcol = c * CW
            rec4 = tmp.tile([P, 4], bf16, name="rec4", tag="rec4", bufs=4)
            anrm = tmp.tile([P, 4, HD], bf16, name="anrm", tag="anrm", bufs=4)
            pT = fil.tile([P, 264], bf16, name="pT", tag="fil")
            for b in range(4):
                nc.tensor.transpose(
                    pT[:, 66 * b:66 * b + 65],
                    pvs_sb[0:HD + 1, h, 128 * b:128 * (b + 1)],
                    ida_sb[0:HD + 1, 0:HD + 1])
            pT3 = pT.rearrange("p (b c) -> p b c", c=66)
            with nc.allow_low_precision(reason="bf16 softmax denom recip"):
                nc.vector.reciprocal(rec4[:, :], pT3[:, :, 0])
            nc.vector.tensor_mul(
                anrm[:, :, :], pT3[:, :, 1:HD + 1],
                rec4[:, :, None].broadcast_to([P, 4, HD]))
            pout = fil.tile([P, CW], bf16, name="pout", tag="fil")
            tpos = (0, 0) if h == 0 else (0, HD)
            rows = slice(0, HD) if h == 0 else slice(HD, P)
            for b in range(4):
                nc.tensor.transpose(
                    pout[rows, 128 * b:128 * (b + 1)],
                    anrm[:, b, :], ida_sb[:, :], tile_position=tpos)
            nc.vector.tensor_copy(attnT[hp][rows, gcol:gcol + CW],
                                  pout[rows, 0:CW])
        return f

    first = True
    sc_cur = lo_cur = None
    deferred = []
    for hp in range(2):
        for c in range(NCH):
            jmax = 4 * c + 3
            base_fillers, budget, ins = sched[(hp, c)]
            # deferred completion drains early (hp1 wo tasks read its attnT
            # output) but AFTER this chunk's kv/krep chain so the scores it
            # feeds are never displaced
            fillers = base_fillers[:ins] + deferred + base_fillers[ins:]
            deferred = []
            fi = 0
            if first:
                sc_cur, lo_cur = scores(0, 0, 0)
                first = False
            pvE = pvp.tile([P, CW], f32, name="pvE", tag="pv")
            pvO = pvp.tile([P, CW], f32, name="pvO", tag="pv")
            for j in range(jmax + 1):
                sc, lo = sc_cur, lo_cur
                pt = ptp.tile([P, 2, CW], bf16, name="pt", tag="pt")
                nc.scalar.activation(pt[:, :, lo:CW], sc[:, :, lo:CW],
                                     AF.Exp, scale=0.125)
                for _ in range(budget):
                    if fi < len(fillers):
                        fillers[fi]()
                        fi += 1
                if j < jmax:
                    sc_cur, lo_cur = scores(hp, c, j + 1)
                st, sp = (j == 0), (j == jmax)
                nc.tensor.matmul(pvE[0:HD + 1, lo:CW], v_sb[:, j, :],
                                 pt[:, 0, lo:CW], start=st, stop=sp)
                nc.tensor.matmul(pvO[0:HD + 1, lo:CW], v_sb[:, j, :],
                                 pt[:, 1, lo:CW], start=st, stop=sp)
            while fi < len(fillers):
                fillers[fi]()
                fi += 1
            # next chunk's first scores ahead of the completion chain
            if (hp, c) != (1, NCH - 1):
                nhp, ncc = (hp, c + 1) if c < NCH - 1 else (hp + 1, 0)
                sc_cur, lo_cur = scores(nhp, ncc, 0)
            # ---- chunk complete: evict pv to SBUF inline (the only pv
            # readers), then DEFER the transpose-normalize work into the
            # next chunk's filler stream so it never serializes the
            # inter-chunk PE FIFO
            pvs_sb = tmp.tile([P, 2, CW], bf16, name="pvs", tag="pvs", bufs=2)
            nc.vector.tensor_copy(pvs_sb[0:HD + 1, 0, :], pvE[0:HD + 1, :])
            nc.vector.tensor_copy(pvs_sb[0:HD + 1, 1, :], pvO[0:HD + 1, :])
            if (hp, c) != (1, NCH - 1):
                deferred = [comp_h(hp, c, 0, pvs_sb), comp_h(hp, c, 1, pvs_sb)]
            else:
                comp_h(hp, c, 0, pvs_sb)()
                comp_h(hp, c, 1, pvs_sb)()

    # ---- tail: only the attnT[1]-half of the last chunk's Wo remains ----
    for tt in range(12, 16):
        for e in range(2):
            for t in wo_late(tt, e):
                t()


def _host_inputs(X, cos, sin, Wq, Wk, Wv, Wo):
    """Build the 8 per-core input maps (host-side sharding + layout prep)."""
    cosT = np.ascontiguousarray(cos.T)  # [64, 2048]
    sinT = np.ascontiguousarray(sin.T)
    cost = np.concatenate([cosT, cosT], 0).astype(BF16)  # [128, 2048]
    sint = np.concatenate([sinT, sinT], 0).astype(BF16)
    rott = np.zeros((P, P), np.float32)
    idx = np.arange(0, P, 2)
    rott[idx, idx + 1] = 1.0    # RT[2i, 2i+1] = +1
    rott[idx + 1, idx] = -1.0   # RT[2i+1, 2i] = -1
    rott = rott.astype(BF16)
    ident = np.zeros((P, P), np.float32)
    ident[0:64, 0:64] = np.eye(64)
    ident[64:128, 0:64] = np.eye(64)   # same I64 available at base partition 64
    ident = ident.astype(BF16)
    # strictly-lower-triangular -60000: added into scores before exp so the
    # upper-left (tk > tq) of each diagonal block becomes exp(-inf) = 0
    ltri = np.tril(np.full((P, P), -60000.0, np.float32), k=-1).astype(BF16)
    ida = np.eye(P, dtype=np.float32).astype(BF16)

    # xt host layout [p, chunk, ct, t]: 8KB contiguous per (p, chunk) DMA run
    xts = [
        np.ascontiguousarray(
            X[b].T.astype(BF16).reshape(8, P, 4, CW).transpose(1, 2, 0, 3))
        for b in range(X.shape[0])
    ]

    in_maps = []
    for c in range(8):
        b, g = c // 4, c % 4
        wqkv = np.concatenate(
            [Wq[256 * g:256 * (g + 1)], Wk[64 * g:64 * (g + 1)], Wv[64 * g:64 * (g + 1)]], 0
        ).T.astype(BF16)                                   # [1024, 384]
        # wqkv host layout [p, ct, d]: 6KB contiguous per partition
        wqkv = np.ascontiguousarray(wqkv.reshape(8, P, QKV).transpose(1, 0, 2))
        wog = np.ascontiguousarray(Wo[:, 256 * g:256 * (g + 1)].T).astype(BF16)  # [256, 1024]
        in_maps.append({
            "xt": xts[b], "wqkv": wqkv, "wo": wog,
            "cost": cost, "sint": sint, "rott": rott, "ident": ident,
            "ltri": ltri, "ida": ida,
        })
    return in_maps


def get_nc():
    if "nc" not in _NC_CACHE:
        _NC_CACHE["nc"] = _build_kernel_program()
    return _NC_CACHE["nc"]


def _install_ntff_hook():
    """The agent image's antenv lacks axon_hooks; recreate it so trace=True
    can reach the terminal's NRT profiler (timing only, not needed for
    correctness)."""
    import types
    if "antenv.axon_hooks" in sys.modules:
        return
    try:
        import antenv
        m = types.ModuleType("antenv.axon_hooks")
        holder = {"v": None}
        m.set_axon_ntff_profile_hook = lambda h: holder.__setitem__("v", h)
        m.get_axon_ntff_profile_hook = lambda: holder["v"]
        sys.modules["antenv.axon_hooks"] = m
        antenv.axon_hooks = m
        from trn_agent_boot.trn_boot import _ntff_profile_via_ctypes
        m.set_axon_ntff_profile_hook(
            _ntff_profile_via_ctypes("/opt/axon/libaxon_pjrt.so"))
    except Exception:
        pass


def kernel(X, freqs_cos, freqs_sin, Wq, Wk, Wv, Wo, _trace=False):
    from concourse.bass_utils import run_bass_kernel_spmd

    if _trace:
        _install_ntff_hook()

    X = np.asarray(X, np.float32)
    in_maps = _host_inputs(
        X, np.asarray(freqs_cos, np.float32), np.asarray(freqs_sin, np.float32),
        np.asarray(Wq, np.float32), np.asarray(Wk, np.float32),
        np.asarray(Wv, np.float32), np.asarray(Wo, np.float32),
    )
    nc = get_nc()
    res = run_bass_kernel_spmd(nc, in_maps, core_ids=list(range(8)), trace=_trace)
    out = np.zeros((2, S, DIM), np.float32)
    for c in range(8):
        o = res.results[c]["outp"].astype(np.float32)   # [16, 2, 128, 512]
        out[c // 4] += o.transpose(0, 2, 1, 3).reshape(S, DIM)
    if _trace:
        kernel.last_result = res
    return out

